# revision 22
# baseline (speedup 1.0000x reference)
"""MoE expert MLP (SwiGLU, top-2 routing) on 8 Trainium2 NeuronCores.

Strategy: expert-parallel. Host routes tokens (stable argsort by expert id,
matching the reference), gathers each expert's token rows, and pads them to a
fixed capacity C. Core e runs expert e's two GEMMs + SwiGLU over its C-column
token panel; the host scatters results back into the permuted [N, H] output.

Design (trace-driven; ~49.5us max-core vs the 58.3us session baseline; PE clock throttles 69->83ns/matmul after sustained load, recovering after ~5min idle):
  - All per-core input bytes stream in exact PE consumption order on the
    sync HWDGE ring at ~420 GB/s: [xT+a0 (pre-Tile) | b0 | w1 pairs 1-7 |
    a8..a15 + fp8 b8..b15 | w2 fp8]. Per-chunk column-slice DMAs into
    persistent tiles give Tile-tracked gating with no buffer rotation.
  - The PE's first dependency (xT + a0's first 4 k-tiles) is its own small
    pre-Tile DMA gated on the tensor engine, so the first matmul starts as
    soon as ~0.41MB lands; a0 k4-7 + b0 ride as the first Tile chunk
    (Tile gates the a-chain mid-accumulation, which is legal).
  - Quantization (e3m4 = TRN FP8_EXP3, 4 mantissa bits): all of w2 and the
    b-halves of pairs 6-15 (NFP8B=10) are stored *64 in fp8, cutting 3.4MB
    (~26%) off the stream. The 1/64 compensation folds into the bf16
    b-halves (exponent shift, lossless) and a 2^-12 scalar on sa for the
    fp8-b pairs. Measured end-to-end rel err 0.01804 vs the 0.02 gate
    (numpy-predicted 0.01796; NFP8B=12 -> 0.0186, 14 -> 0.0192, full w1
    breaches). fp8 stationary matmuls cost the same 69ns as bf16.
  - C=144 (max expert load for this routing is 142).
  - Known residual costs on the max core: ~6.8us framework preamble before
    the first DMA descriptor, and each core's slowest SDMA engine (~21 vs
    26 GB/s) pacing full-chunk semaphores (~336 GB/s effective). Progressive
    GEMM2 variants were tried and rejected: under straggler pacing the pair
    loop has zero PE slack, so moving GEMM2 work into it cannot win.

Per-core dataflow (all activations transposed, tokens on the free dim):
  GEMM1:  h1T[m-block j] = w1[e][:, cols].T @ xT      (8 H k-tiles, PSUM acc)
  SwiGLU: interT[j] = silu(h1T_a[j]) * h1T_b[j]       (ACT + DVE out of PSUM)
  GEMM2:  yT[h-block]  = w2q[e][:, cols].T @ interT   (16 I k-tiles, fp8 lhsT)
"""

import numpy as np
import ml_dtypes

import concourse.bass as bass
import concourse.mybir as mybir
import concourse.tile as tile
from concourse import bacc
from concourse.bass_utils import run_bass_kernel_spmd

BF16 = mybir.dt.bfloat16
FP8 = mybir.dt.float8e3
F32 = mybir.dt.float32
NP_BF16 = ml_dtypes.bfloat16
NP_FP8 = ml_dtypes.float8_e3m4

# Problem shape (hardcoded per the contract; matches nn_Experts_41429254537622)
B, S, H, I, E, TOPK = 1, 512, 1024, 2048, 8, 2
N_CORES = 8
KH = H // 128    # 8  k-tiles for GEMM1 (contraction over H)
NPAIR = I // 128 # 16 (a, b) pairs of 128-wide w1 column blocks
KI = I // 128    # 16 k-tiles for GEMM2 (contraction over I)
MH = H // 128    # 8  output row blocks of yT
PAIR_COLS = 2 * KH * 128   # 2048 w1 blob columns per (a, b) pair block
HB_COLS = KI * 128         # 2048 w2 blob columns per h block (fp8: 1B each)
W2_SCALE = 64.0            # w2 stored as e3m4 * 64; 1/64 folded into w1's b half
NFP8B = 12                 # trailing pairs whose b half is stored fp8 (b*64)
BQ0 = NPAIR - NFP8B        # first fp8-b pair
BQ_COLS = KH * 128         # 1024 fp8 b columns per pair

_compiled = {}
LAST_RUNS = []  # BassKernelResults of the most recent kernel() call (for test harness)


def _build_program(C):
    XCOLS = KH * C
    nc = bacc.Bacc(
        "TRN2", target_bir_lowering=False, debug=False, num_devices=N_CORES
    )
    # blob1: [ xT (XCOLS) | pair0 a+b (PAIR_COLS) | pairs 1..15 ] in exact
    # consumption order. blob2: w2 as fp8, h-block-major.
    blob1 = nc.dram_tensor(
        "blob1",
        [128, XCOLS + BQ0 * PAIR_COLS + NFP8B * BQ_COLS],
        BF16,
        kind="ExternalInput",
    )
    blob2 = nc.dram_tensor(
        "blob2", [128, NFP8B * BQ_COLS + MH * HB_COLS], FP8,
        kind="ExternalInput",
    )
    yT_d = nc.dram_tensor("yT", [128, MH * C], BF16, kind="ExternalOutput")

    # Pre-Tile raw loads: the PE's first work (pair 0's a-chain) needs only
    # xT + a0, so that prefix is its own DMA; b0 follows. Both stream during
    # the framework preamble; the consumer-side wait sits on the tensor
    # engine, program-order ahead of every Tile-emitted PE instruction.
    P0A = XCOLS + 4 * 128           # end of [xT | a0 k0-3]
    P0B = XCOLS + PAIR_COLS         # end of [xT | a0 | b0]
    xw0_raw = nc.alloc_sbuf_tensor("xw0_pre", [128, P0A], BF16)
    pre_sem = nc.alloc_semaphore(name="pre_dma_sem")
    xw0 = xw0_raw.ap()
    nc.sync.dma_start(xw0[:, :P0A], blob1[:, :P0A]).then_inc(pre_sem, 16)
    nc.tensor.wait_ge(pre_sem, 16)
    nc.tensor.sem_clear(pre_sem)
    xt = xw0[:, :XCOLS]

    with tile.TileContext(nc) as tc:
        with (
            tc.tile_pool(name="wp", bufs=1) as wp,
            tc.tile_pool(name="sap", bufs=4) as sap,
            tc.tile_pool(name="outp", bufs=2) as outp,
            tc.tile_pool(name="ps1", bufs=6, space="PSUM") as ps1,
            tc.tile_pool(name="ps2", bufs=2, space="PSUM") as ps2,
        ):
            # Persistent weight panels + inter panel: DMAs write disjoint
            # column slices, matmuls read sub-slices; Tile's range tracker
            # gives per-chunk gating with no buffer rotation or WAR stalls.
            # w1t holds [pair0-b | pairs 1..7 full | a8..a15]; b8..b15
            # live fp8 in w1bq (stored *64, compensated by 2^-12 on sa).
            B0 = PAIR_COLS - 4 * 128    # [a0 k4-7 | b0] head of w1t
            w1t = wp.tile(
                [128, B0 + (BQ0 - 1) * PAIR_COLS + NFP8B * BQ_COLS], BF16
            )
            w1bq = wp.tile([128, NFP8B * BQ_COLS], FP8)
            w2t = wp.tile([128, MH * HB_COLS], FP8)
            it_all = wp.tile([128, KI * C], BF16)
            A8 = B0 + (BQ0 - 1) * PAIR_COLS     # w1t col of a8

            # b0, w1 pairs 1..7 (1MB chunks of 2), then per 2 fp8-b pairs an
            # a-chunk (bf16) + b-chunk (fp8), then w2; back-to-back on sync.
            nc.sync.dma_start(w1t[:, :B0], blob1[:, P0A:P0B])
            for p0 in range(1, BQ0, 2):
                p1 = min(p0 + 2, BQ0)
                nc.sync.dma_start(
                    w1t[:, B0 + (p0 - 1) * PAIR_COLS:B0 + (p1 - 1) * PAIR_COLS],
                    blob1[:, XCOLS + p0 * PAIR_COLS:XCOLS + p1 * PAIR_COLS],
                )
            AB = XCOLS + BQ0 * PAIR_COLS        # blob1 col of a8
            for jj in range(0, NFP8B, 2):
                nc.sync.dma_start(
                    w1t[:, A8 + jj * BQ_COLS:A8 + (jj + 2) * BQ_COLS],
                    blob1[:, AB + jj * BQ_COLS:AB + (jj + 2) * BQ_COLS],
                )
                nc.sync.dma_start(
                    w1bq[:, jj * BQ_COLS:(jj + 2) * BQ_COLS],
                    blob2[:, jj * BQ_COLS:(jj + 2) * BQ_COLS],
                )
            W2B = NFP8B * BQ_COLS               # blob2 col of w2
            for hc in range(MH):
                nc.sync.dma_start(
                    w2t[:, hc * HB_COLS:(hc + 1) * HB_COLS],
                    blob2[:, W2B + hc * HB_COLS:W2B + (hc + 1) * HB_COLS],
                )

            # GEMM1 + SwiGLU, pair-by-pair in stream order.
            for j in range(NPAIR):
                if j == 0:
                    asrc, abase = None, 0         # per-k split below
                    bsrc, bbase = w1t, -KH * 128 + 4 * 128  # b0 at w1t[512:1536]
                elif j < BQ0:
                    asrc, abase = w1t, B0 + (j - 1) * PAIR_COLS
                    bsrc, bbase = w1t, B0 + (j - 1) * PAIR_COLS
                else:
                    asrc, abase = w1t, A8 + (j - BQ0) * BQ_COLS
                    bsrc, bbase = w1bq, (j - BQ0) * BQ_COLS - KH * 128
                pa = ps1.tile([128, C], F32, tag="pab")
                pb = ps1.tile([128, C], F32, tag="pab")
                for k in range(KH):
                    if j == 0:
                        if k < 4:
                            ak = xw0[:, XCOLS + k * 128:XCOLS + (k + 1) * 128]
                        else:
                            ak = w1t[:, (k - 4) * 128:(k - 3) * 128]
                    else:
                        ak = asrc[:, abase + k * 128:abase + (k + 1) * 128]
                    nc.tensor.matmul(
                        pa[:],
                        ak,
                        xt[:, k * C:(k + 1) * C],
                        start=(k == 0),
                        stop=(k == KH - 1),
                    )
                for k in range(KH):
                    nc.tensor.matmul(
                        pb[:],
                        bsrc[:, bbase + (KH + k) * 128:bbase + (KH + k + 1) * 128],
                        xt[:, k * C:(k + 1) * C],
                        start=(k == 0),
                        stop=(k == KH - 1),
                    )
                sa = sap.tile([128, C], F32, tag="sa")
                nc.scalar.activation(
                    sa[:], pa[:], mybir.ActivationFunctionType.Silu
                )
                if j >= BQ0:
                    # b was stored *64 instead of /64: fold 2^-12 into sa.
                    nc.vector.tensor_scalar_mul(sa[:], sa[:], 2.0 ** -12)
                nc.vector.tensor_mul(it_all[:, j * C:(j + 1) * C], sa[:], pb[:])

            # GEMM2 with fp8 stationary tiles; store every 2 h-blocks on the
            # scalar ring so stores never head-block the weight stream.
            for hc in range(0, MH, 2):
                yt = outp.tile([128, 2 * C], BF16, tag="yt")
                for hh in range(2):
                    h = hc + hh
                    py = ps2.tile([128, C], F32, tag="py")
                    for ki in range(KI):
                        nc.tensor.matmul(
                            py[:],
                            w2t[:, h * HB_COLS + ki * 128:h * HB_COLS + (ki + 1) * 128],
                            it_all[:, ki * C:(ki + 1) * C],
                            start=(ki == 0),
                            stop=(ki == KI - 1),
                        )
                    nc.vector.tensor_copy(yt[:, hh * C:(hh + 1) * C], py[:])
                nc.scalar.dma_start(
                    yT_d[:, hc * C:(hc + 2) * C], yt[:]
                )
    nc.compile()
    return nc


def _get_program(C):
    if C not in _compiled:
        _compiled[C] = _build_program(C)
    return _compiled[C]


def _relayout_w1(w1_e):
    # w1_e: [H, 2I] bf16 (b-half of pairs < BQ0 pre-scaled by 1/64) ->
    # [128, BQ0*PAIR_COLS + NFP8B*BQ_COLS]: pairs 0..BQ0-1 hold a_j's 8
    # k-tiles then b_j's; pairs BQ0.. hold only a_j (their b is fp8 in
    # blob2). Stationary [K=128, M=128] layout (partition = contraction row).
    A = w1_e[:, :I].reshape(KH, 128, NPAIR, 128)
    Bh = w1_e[:, I:].reshape(KH, 128, NPAIR, 128)
    pairs = np.stack([A[:, :, :BQ0], Bh[:, :, :BQ0]], axis=0)
    full = pairs.transpose(2, 3, 0, 1, 4).reshape(128, BQ0 * PAIR_COLS)
    atail = A[:, :, BQ0:].transpose(1, 2, 0, 3).reshape(128, NFP8B * BQ_COLS)
    return np.ascontiguousarray(np.concatenate([full, atail], axis=1))


def _relayout_w1bq(bq_e):
    # bq_e: [H, NFP8B*128] fp8 (*64) -> [128, NFP8B*BQ_COLS], per-pair the
    # 8 k-tiles in stationary layout.
    r = bq_e.reshape(KH, 128, NFP8B, 128)
    return np.ascontiguousarray(
        r.transpose(1, 2, 0, 3).reshape(128, NFP8B * BQ_COLS)
    )


def _relayout_w2(w2_e):
    # w2_e: [I, H] fp8 -> [128, MH*HB_COLS], h-block-major: h block holds its
    # KI stationary k-tiles in consumption order.
    r = w2_e.reshape(KI, 128, MH, 128)
    return np.ascontiguousarray(
        r.transpose(1, 2, 0, 3).reshape(128, MH * HB_COLS)
    )


def kernel(hidden_states, tokens_per_expert, w1, w2):
    x = np.asarray(hidden_states).reshape(-1, H)
    flat = np.asarray(tokens_per_expert).reshape(-1).astype(np.int64)
    w1 = np.asarray(w1)
    w2 = np.asarray(w2)
    n_rows = flat.shape[0]

    order = np.argsort(flat, kind="stable")
    token_of_row = order // TOPK
    counts = np.bincount(flat, minlength=E)
    starts = np.concatenate([[0], np.cumsum(counts)[:-1]])

    x_bf = x.astype(NP_BF16)
    if w1.dtype != NP_BF16:
        w1 = w1.astype(NP_BF16)

    C = max(48, int(-(-int(counts.max()) // 2)) * 2)
    XCOLS = KH * C
    nc = _get_program(C)

    # b-half of bf16 pairs scaled by 1/W2_SCALE (exponent shift, lossless);
    # w2 and the trailing b-halves stored as e3m4 * W2_SCALE (the latter
    # compensated by 2^-12 on sa in-kernel).
    w1s = np.concatenate(
        [w1[:, :, :I], (w1[:, :, I:].astype(np.float32) / W2_SCALE).astype(NP_BF16)],
        axis=2,
    )
    bq = (
        w1[:, :, I + BQ0 * 128:].astype(np.float32) * W2_SCALE
    ).astype(NP_FP8)
    w2q = (w2.astype(np.float32) * W2_SCALE).astype(NP_FP8)
    w1r = [_relayout_w1(w1s[e]) for e in range(E)]
    bqr = [_relayout_w1bq(bq[e]) for e in range(E)]
    w2r = [_relayout_w2(w2q[e]) for e in range(E)]

    out = np.zeros((n_rows, H), dtype=NP_BF16)
    LAST_RUNS.clear()
    n_waves = int(max(1, -(-int(counts.max()) // C)))
    for wave in range(n_waves):
        in_maps = []
        for e in range(E):
            lo = starts[e] + wave * C
            cnt = int(min(C, max(0, counts[e] - wave * C)))
            xe = np.zeros((C, H), dtype=NP_BF16)
            if cnt:
                xe[:cnt] = x_bf[token_of_row[lo:lo + cnt]]
            # xT layout: [128, KH*C], k-tile k at cols [k*C, (k+1)*C):
            # xT[p, k*C + c] = xe[c, k*128 + p]
            xT = np.ascontiguousarray(
                xe.T.reshape(KH, 128, C).transpose(1, 0, 2).reshape(128, XCOLS)
            )
            blob1 = np.concatenate([xT, w1r[e]], axis=1)
            blob2 = np.concatenate([bqr[e], w2r[e]], axis=1)
            in_maps.append({"blob1": blob1, "blob2": blob2})

        res = run_bass_kernel_spmd(nc, in_maps, list(range(N_CORES)))
        LAST_RUNS.append(res)
        for e in range(E):
            lo = starts[e] + wave * C
            cnt = int(min(C, max(0, counts[e] - wave * C)))
            if not cnt:
                continue
            yT = res.results[e]["yT"]
            # yT[p, h*C + c] = y[c, h*128 + p]
            y = yT.reshape(128, MH, C).transpose(2, 1, 0).reshape(C, H)
            out[lo:lo + cnt] = y[:cnt]
    return out


# revision 23
# speedup vs baseline: 1.1536x; 1.1536x over previous
"""MoE expert MLP (SwiGLU, top-2 routing) on 8 Trainium2 NeuronCores.

Strategy: expert-parallel. Host routes tokens (stable argsort by expert id,
matching the reference), gathers each expert's token rows, and pads them to a
fixed capacity C. Core e runs expert e's two GEMMs + SwiGLU over its C-column
token panel; the host scatters results back into the permuted [N, H] output.

Design (trace-driven; ~49.5us max-core vs the 58.3us session baseline; PE clock throttles 69->83ns/matmul after sustained load, recovering after ~5min idle):
  - All per-core input bytes stream in exact PE consumption order on the
    sync HWDGE ring at ~420 GB/s: [xT+a0 (pre-Tile) | b0 | w1 full pairs |
    a-tail + fp8 b-tail | w2 fp8]. Per-chunk column-slice DMAs into
    persistent tiles give Tile-tracked gating with no buffer rotation.
  - The PE's first dependency (xT + a0's first 4 k-tiles) is its own small
    pre-Tile DMA gated on the tensor engine, so the first matmul starts as
    soon as ~0.41MB lands; a0 k4-7 + b0 ride as the first Tile chunk
    (Tile gates the a-chain mid-accumulation, which is legal).
  - Quantization (e3m4 = TRN FP8_EXP3, 4 mantissa bits): all of w2 and the
    b-halves of pairs 4-15 (NFP8B=12) are stored *64 in fp8, cutting 3.7MB
    (~28%) off the stream. The 1/64 compensation folds into the bf16
    b-halves (exponent shift, lossless) and a 2^-12 scalar on sa for the
    fp8-b pairs. Measured end-to-end rel err 0.018656 vs the 0.02 gate
    (numpy predicts 0.018582; HW adds a stable +8e-5. NFP8B=14 -> 0.0193
    leaves <4% margin - declined; full w1 breaches). fp8 stationary
    matmuls cost the same 69ns as bf16.
  - C=142 = the exact max expert load for this routing (ceil to mult of 2).
  - Known residual costs on the max core: ~6.8us framework preamble before
    the first DMA descriptor, and each core's slowest SDMA engine (~21 vs
    26 GB/s) pacing full-chunk semaphores (~336 GB/s effective). Progressive
    GEMM2 variants were tried and rejected: under straggler pacing the pair
    loop has zero PE slack, so moving GEMM2 work into it cannot win.

Per-core dataflow (all activations transposed, tokens on the free dim):
  GEMM1:  h1T[m-block j] = w1[e][:, cols].T @ xT      (8 H k-tiles, PSUM acc)
  SwiGLU: interT[j] = silu(h1T_a[j]) * h1T_b[j]       (ACT + DVE out of PSUM)
  GEMM2:  yT[h-block]  = w2q[e][:, cols].T @ interT   (16 I k-tiles, fp8 lhsT)
"""

import numpy as np
import ml_dtypes

import concourse.bass as bass
import concourse.mybir as mybir
import concourse.tile as tile
from concourse import bacc
from concourse.bass_utils import run_bass_kernel_spmd

BF16 = mybir.dt.bfloat16
FP8 = mybir.dt.float8e3
F32 = mybir.dt.float32
NP_BF16 = ml_dtypes.bfloat16
NP_FP8 = ml_dtypes.float8_e3m4

# Problem shape (hardcoded per the contract; matches nn_Experts_41429254537622)
B, S, H, I, E, TOPK = 1, 512, 1024, 2048, 8, 2
N_CORES = 8
KH = H // 128    # 8  k-tiles for GEMM1 (contraction over H)
NPAIR = I // 128 # 16 (a, b) pairs of 128-wide w1 column blocks
KI = I // 128    # 16 k-tiles for GEMM2 (contraction over I)
MH = H // 128    # 8  output row blocks of yT
PAIR_COLS = 2 * KH * 128   # 2048 w1 blob columns per (a, b) pair block
HB_COLS = KI * 128         # 2048 w2 blob columns per h block (fp8: 1B each)
W2_SCALE = 64.0            # w2 stored as e3m4 * 64; 1/64 folded into w1's b half
NFP8B = 12                 # trailing pairs whose b half is stored fp8 (b*64)
BQ0 = NPAIR - NFP8B        # first fp8-b pair
BQ_COLS = KH * 128         # 1024 fp8 b columns per pair

_compiled = {}
LAST_RUNS = []  # BassKernelResults of the most recent kernel() call (for test harness)


def _build_program(C):
    XCOLS = KH * C
    nc = bacc.Bacc(
        "TRN2", target_bir_lowering=False, debug=False, num_devices=N_CORES
    )
    # blob1: [ xT (XCOLS) | pair0 a+b (PAIR_COLS) | pairs 1..15 ] in exact
    # consumption order. blob2: w2 as fp8, h-block-major.
    blob1 = nc.dram_tensor(
        "blob1",
        [128, XCOLS + BQ0 * PAIR_COLS + NFP8B * BQ_COLS],
        BF16,
        kind="ExternalInput",
    )
    blob2 = nc.dram_tensor(
        "blob2", [128, NFP8B * BQ_COLS + MH * HB_COLS], FP8,
        kind="ExternalInput",
    )
    yT_d = nc.dram_tensor("yT", [128, MH * C], BF16, kind="ExternalOutput")

    # Pre-Tile raw loads: the PE's first work (pair 0's a-chain) needs only
    # xT + a0, so that prefix is its own DMA; b0 follows. Both stream during
    # the framework preamble; the consumer-side wait sits on the tensor
    # engine, program-order ahead of every Tile-emitted PE instruction.
    P0A = XCOLS + 4 * 128           # end of [xT | a0 k0-3]
    P0B = XCOLS + PAIR_COLS         # end of [xT | a0 | b0]
    xw0_raw = nc.alloc_sbuf_tensor("xw0_pre", [128, P0A], BF16)
    pre_sem = nc.alloc_semaphore(name="pre_dma_sem")
    xw0 = xw0_raw.ap()
    nc.sync.dma_start(xw0[:, :P0A], blob1[:, :P0A]).then_inc(pre_sem, 16)
    nc.tensor.wait_ge(pre_sem, 16)
    nc.tensor.sem_clear(pre_sem)
    xt = xw0[:, :XCOLS]

    with tile.TileContext(nc) as tc:
        with (
            tc.tile_pool(name="wp", bufs=1) as wp,
            tc.tile_pool(name="sap", bufs=4) as sap,
            tc.tile_pool(name="outp", bufs=2) as outp,
            tc.tile_pool(name="ps1", bufs=6, space="PSUM") as ps1,
            tc.tile_pool(name="ps2", bufs=2, space="PSUM") as ps2,
        ):
            # Persistent weight panels + inter panel: DMAs write disjoint
            # column slices, matmuls read sub-slices; Tile's range tracker
            # gives per-chunk gating with no buffer rotation or WAR stalls.
            # w1t holds [pair0-b | pairs 1..7 full | a8..a15]; b8..b15
            # live fp8 in w1bq (stored *64, compensated by 2^-12 on sa).
            B0 = PAIR_COLS - 4 * 128    # [a0 k4-7 | b0] head of w1t
            w1t = wp.tile(
                [128, B0 + (BQ0 - 1) * PAIR_COLS + NFP8B * BQ_COLS], BF16
            )
            w1bq = wp.tile([128, NFP8B * BQ_COLS], FP8)
            w2t = wp.tile([128, MH * HB_COLS], FP8)
            it_all = wp.tile([128, KI * C], BF16)
            A8 = B0 + (BQ0 - 1) * PAIR_COLS     # w1t col of a8

            # b0, w1 pairs 1..7 (1MB chunks of 2), then per 2 fp8-b pairs an
            # a-chunk (bf16) + b-chunk (fp8), then w2; back-to-back on sync.
            nc.sync.dma_start(w1t[:, :B0], blob1[:, P0A:P0B])
            for p0 in range(1, BQ0, 2):
                p1 = min(p0 + 2, BQ0)
                nc.sync.dma_start(
                    w1t[:, B0 + (p0 - 1) * PAIR_COLS:B0 + (p1 - 1) * PAIR_COLS],
                    blob1[:, XCOLS + p0 * PAIR_COLS:XCOLS + p1 * PAIR_COLS],
                )
            AB = XCOLS + BQ0 * PAIR_COLS        # blob1 col of a8
            for jj in range(0, NFP8B, 2):
                nc.sync.dma_start(
                    w1t[:, A8 + jj * BQ_COLS:A8 + (jj + 2) * BQ_COLS],
                    blob1[:, AB + jj * BQ_COLS:AB + (jj + 2) * BQ_COLS],
                )
                nc.sync.dma_start(
                    w1bq[:, jj * BQ_COLS:(jj + 2) * BQ_COLS],
                    blob2[:, jj * BQ_COLS:(jj + 2) * BQ_COLS],
                )
            W2B = NFP8B * BQ_COLS               # blob2 col of w2
            for hc in range(MH):
                nc.sync.dma_start(
                    w2t[:, hc * HB_COLS:(hc + 1) * HB_COLS],
                    blob2[:, W2B + hc * HB_COLS:W2B + (hc + 1) * HB_COLS],
                )

            # GEMM1 + SwiGLU, pair-by-pair in stream order.
            for j in range(NPAIR):
                if j == 0:
                    asrc, abase = None, 0         # per-k split below
                    bsrc, bbase = w1t, -KH * 128 + 4 * 128  # b0 at w1t[512:1536]
                elif j < BQ0:
                    asrc, abase = w1t, B0 + (j - 1) * PAIR_COLS
                    bsrc, bbase = w1t, B0 + (j - 1) * PAIR_COLS
                else:
                    asrc, abase = w1t, A8 + (j - BQ0) * BQ_COLS
                    bsrc, bbase = w1bq, (j - BQ0) * BQ_COLS - KH * 128
                pa = ps1.tile([128, C], F32, tag="pab")
                pb = ps1.tile([128, C], F32, tag="pab")
                for k in range(KH):
                    if j == 0:
                        if k < 4:
                            ak = xw0[:, XCOLS + k * 128:XCOLS + (k + 1) * 128]
                        else:
                            ak = w1t[:, (k - 4) * 128:(k - 3) * 128]
                    else:
                        ak = asrc[:, abase + k * 128:abase + (k + 1) * 128]
                    nc.tensor.matmul(
                        pa[:],
                        ak,
                        xt[:, k * C:(k + 1) * C],
                        start=(k == 0),
                        stop=(k == KH - 1),
                    )
                for k in range(KH):
                    nc.tensor.matmul(
                        pb[:],
                        bsrc[:, bbase + (KH + k) * 128:bbase + (KH + k + 1) * 128],
                        xt[:, k * C:(k + 1) * C],
                        start=(k == 0),
                        stop=(k == KH - 1),
                    )
                sa = sap.tile([128, C], F32, tag="sa")
                nc.scalar.activation(
                    sa[:], pa[:], mybir.ActivationFunctionType.Silu
                )
                if j >= BQ0:
                    # b was stored *64 instead of /64: fold 2^-12 into sa.
                    nc.vector.tensor_scalar_mul(sa[:], sa[:], 2.0 ** -12)
                nc.vector.tensor_mul(it_all[:, j * C:(j + 1) * C], sa[:], pb[:])

            # GEMM2 with fp8 stationary tiles; store every 2 h-blocks on the
            # scalar ring so stores never head-block the weight stream.
            for hc in range(0, MH, 2):
                yt = outp.tile([128, 2 * C], BF16, tag="yt")
                for hh in range(2):
                    h = hc + hh
                    py = ps2.tile([128, C], F32, tag="py")
                    for ki in range(KI):
                        nc.tensor.matmul(
                            py[:],
                            w2t[:, h * HB_COLS + ki * 128:h * HB_COLS + (ki + 1) * 128],
                            it_all[:, ki * C:(ki + 1) * C],
                            start=(ki == 0),
                            stop=(ki == KI - 1),
                        )
                    nc.vector.tensor_copy(yt[:, hh * C:(hh + 1) * C], py[:])
                nc.scalar.dma_start(
                    yT_d[:, hc * C:(hc + 2) * C], yt[:]
                )
    nc.compile()
    return nc


def _get_program(C):
    if C not in _compiled:
        _compiled[C] = _build_program(C)
    return _compiled[C]


def _relayout_w1(w1_e):
    # w1_e: [H, 2I] bf16 (b-half of pairs < BQ0 pre-scaled by 1/64) ->
    # [128, BQ0*PAIR_COLS + NFP8B*BQ_COLS]: pairs 0..BQ0-1 hold a_j's 8
    # k-tiles then b_j's; pairs BQ0.. hold only a_j (their b is fp8 in
    # blob2). Stationary [K=128, M=128] layout (partition = contraction row).
    A = w1_e[:, :I].reshape(KH, 128, NPAIR, 128)
    Bh = w1_e[:, I:].reshape(KH, 128, NPAIR, 128)
    pairs = np.stack([A[:, :, :BQ0], Bh[:, :, :BQ0]], axis=0)
    full = pairs.transpose(2, 3, 0, 1, 4).reshape(128, BQ0 * PAIR_COLS)
    atail = A[:, :, BQ0:].transpose(1, 2, 0, 3).reshape(128, NFP8B * BQ_COLS)
    return np.ascontiguousarray(np.concatenate([full, atail], axis=1))


def _relayout_w1bq(bq_e):
    # bq_e: [H, NFP8B*128] fp8 (*64) -> [128, NFP8B*BQ_COLS], per-pair the
    # 8 k-tiles in stationary layout.
    r = bq_e.reshape(KH, 128, NFP8B, 128)
    return np.ascontiguousarray(
        r.transpose(1, 2, 0, 3).reshape(128, NFP8B * BQ_COLS)
    )


def _relayout_w2(w2_e):
    # w2_e: [I, H] fp8 -> [128, MH*HB_COLS], h-block-major: h block holds its
    # KI stationary k-tiles in consumption order.
    r = w2_e.reshape(KI, 128, MH, 128)
    return np.ascontiguousarray(
        r.transpose(1, 2, 0, 3).reshape(128, MH * HB_COLS)
    )


def kernel(hidden_states, tokens_per_expert, w1, w2):
    x = np.asarray(hidden_states).reshape(-1, H)
    flat = np.asarray(tokens_per_expert).reshape(-1).astype(np.int64)
    w1 = np.asarray(w1)
    w2 = np.asarray(w2)
    n_rows = flat.shape[0]

    order = np.argsort(flat, kind="stable")
    token_of_row = order // TOPK
    counts = np.bincount(flat, minlength=E)
    starts = np.concatenate([[0], np.cumsum(counts)[:-1]])

    x_bf = x.astype(NP_BF16)
    if w1.dtype != NP_BF16:
        w1 = w1.astype(NP_BF16)

    C = max(48, int(-(-int(counts.max()) // 2)) * 2)
    XCOLS = KH * C
    nc = _get_program(C)

    # b-half of bf16 pairs scaled by 1/W2_SCALE (exponent shift, lossless);
    # w2 and the trailing b-halves stored as e3m4 * W2_SCALE (the latter
    # compensated by 2^-12 on sa in-kernel).
    w1s = np.concatenate(
        [w1[:, :, :I], (w1[:, :, I:].astype(np.float32) / W2_SCALE).astype(NP_BF16)],
        axis=2,
    )
    bq = (
        w1[:, :, I + BQ0 * 128:].astype(np.float32) * W2_SCALE
    ).astype(NP_FP8)
    w2q = (w2.astype(np.float32) * W2_SCALE).astype(NP_FP8)
    w1r = [_relayout_w1(w1s[e]) for e in range(E)]
    bqr = [_relayout_w1bq(bq[e]) for e in range(E)]
    w2r = [_relayout_w2(w2q[e]) for e in range(E)]

    out = np.zeros((n_rows, H), dtype=NP_BF16)
    LAST_RUNS.clear()
    n_waves = int(max(1, -(-int(counts.max()) // C)))
    for wave in range(n_waves):
        in_maps = []
        for e in range(E):
            lo = starts[e] + wave * C
            cnt = int(min(C, max(0, counts[e] - wave * C)))
            xe = np.zeros((C, H), dtype=NP_BF16)
            if cnt:
                xe[:cnt] = x_bf[token_of_row[lo:lo + cnt]]
            # xT layout: [128, KH*C], k-tile k at cols [k*C, (k+1)*C):
            # xT[p, k*C + c] = xe[c, k*128 + p]
            xT = np.ascontiguousarray(
                xe.T.reshape(KH, 128, C).transpose(1, 0, 2).reshape(128, XCOLS)
            )
            blob1 = np.concatenate([xT, w1r[e]], axis=1)
            blob2 = np.concatenate([bqr[e], w2r[e]], axis=1)
            in_maps.append({"blob1": blob1, "blob2": blob2})

        res = run_bass_kernel_spmd(nc, in_maps, list(range(N_CORES)))
        LAST_RUNS.append(res)
        for e in range(E):
            lo = starts[e] + wave * C
            cnt = int(min(C, max(0, counts[e] - wave * C)))
            if not cnt:
                continue
            yT = res.results[e]["yT"]
            # yT[p, h*C + c] = y[c, h*128 + p]
            y = yT.reshape(128, MH, C).transpose(2, 1, 0).reshape(C, H)
            out[lo:lo + cnt] = y[:cnt]
    return out
